# revision 50
# baseline (speedup 1.0000x reference)
"""Trainium2 Bass kernel v4 for nn_ADDLossSoftEncode (Davenport q-method ADD loss).

Data parallel over batch: B=512 -> 64 samples/core on 8 cores.
Partition layout: p = 2*s + half (sample-interleaved halves) so every DMA
is a single full-width [128, *] transfer with >=2KB contiguous runs.

v4 schedule vs v2 baseline (93us):
  - no q-plane deinterleave: all per-pair ops are 1x-mode and
    stride-insensitive, so they read the interleaved f32 staging buffer
    directly (the v2 multi-plane ACT deint was 3.6us/chunk).
  - ISA note: the TRN2 Pool engine has NO TensorScalarPtr support (no
    tensor_scalar / scalar_tensor_tensor); it does run TensorTensor,
    including with stride-0 broadcast operands and mixed bf16*f32 inputs.
  - Gram pairs: 4x DVE STT+accum (fused, 1127ns) + 5x Pool TT product with
    ACT Identity-accum reduction for chunks 0-2 (chunk 2 folds to 512 via
    DVE add first); the LAST chunk runs 6 pairs as inline DVE STT+accum so
    no Pool->DVE->ACT accumulate relay extends the stage-A drain before
    the eigen chain.
  - dual DMA queues: ori + point chunks on SP, sep/gt on the ACT HWDGE
    queue interleaved with the exps (chunk 0 halved for an early start).
  - ACT act-table switch (exp set -> sqrt set) hoisted mid-kernel, off the
    critical tail; |p|^2 squares run on ACT (Square is in the exp set)
    for c0/c1 and as Pool interleaved TT squares for c2/c3.
  - Gram even/odd-half combine via DVE 32x32 stream-transpose -> free-axis
    pair add -> transpose back (~0.4us, vs ~2.5us for broadcast DMAs).
  - eigen chain: 12 trace-normalized squarings on DVE (spectral gaps reach
    0.9984 so the 2^12 power is required); 3 norms spaced for fp32 range.
  - stage C tail: d-chains on DVE STT (c0/c1) and Pool broadcast-TT
    (c2/c3), relu on DVE 4x tensor_scalar, sqrt+accum on ACT.
"""

import sys
from contextlib import ExitStack

import numpy as np

sys.path.insert(0, "/opt/trn_rl_repo")

import concourse.bass as bass
import concourse.tile as tile
from concourse import bacc
from concourse import mybir

F32 = mybir.dt.float32
BF16 = mybir.dt.bfloat16
AX = mybir.AxisListType
OP = mybir.AluOpType
ACT = mybir.ActivationFunctionType

B, K, P = 512, 8192, 4096
NCORES = 8
S = B // NCORES          # 64 samples per core
KH = K // 2              # 4096 per k-half (one partition row)
KC = 1024                # k-chunk width (per row)
NKC = KH // KC           # 4
PH = P // 2              # 2048 points per row
PC = 512                 # point chunk
NPC = PH // PC           # 4

PAIRS = [(0, 0), (0, 1), (0, 2), (0, 3), (1, 1), (1, 2), (1, 3), (2, 2), (2, 3)]
UIDX = {p: n for n, p in enumerate(PAIRS)}
# Per-pair mode. The TRN2 Pool engine has no TensorScalarPtr (STT) support,
# so only these are legal:
#   'D'  = DVE STT+accum (one op, 1127ns)
#   'PA' = Pool TT product + ACT Identity accum-1024 (853 + 1225)
#   'PF' = Pool TT product + DVE fold to 512 + ACT accum-512 (853+327+799)
#          pair:  (0,0) (0,1) (0,2) (0,3) (1,1) (1,2) (1,3) (2,2) (2,3)
PAIR_ENG =       ['D',  'D',  'PA', 'PF', 'D',  'PA', 'PF', 'D',  'PA']
PAIR_ENG_C3 =    ['D',  'D',  'D',  'PA', 'D',  'PA', 'D',  'D',  'PA']
# u-plane engines (u_i = e * q_i, strided read): DVE 1127, Pool 853
U_ENG = ['D', 'P', 'P']
NSQ = 12                 # matrix squarings
# trace-normalize: first before fp32 overflow ((sumE~1.3e4)^4 ~ 3e16), then
# spaced <= 5 apart (entries >= (0.25)^16 ~ 2e-10 between norms)
NORM_AT = frozenset({2, 7, NSQ - 1})
# aacc column layout per quantity: [c0a, c0b, c1, c2, c3]
NAC = 5


def _emit(ctx, tc, sep, ori, gt, pt, out):
    nc = tc.nc
    pool_st = ctx.enter_context(tc.tile_pool(name="st", bufs=1))
    pool_q = ctx.enter_context(tc.tile_pool(name="q", bufs=1))
    pool_u = ctx.enter_context(tc.tile_pool(name="u", bufs=1))
    pool_pr = ctx.enter_context(tc.tile_pool(name="pr", bufs=1))
    pool_tiny = ctx.enter_context(tc.tile_pool(name="tiny", bufs=1))
    pool_pt = ctx.enter_context(tc.tile_pool(name="pt", bufs=1))
    pool_c = ctx.enter_context(tc.tile_pool(name="c", bufs=1))

    sep_v = sep.rearrange("s (h k) -> (s h) k", h=2)           # [128, 4096]
    ori_v = ori.rearrange("s (h k) q -> (s h) (k q)", h=2)     # [128, 4096*4]
    pt_v = pt.rearrange("s (h p) x -> (s h) (p x)", h=2)       # [128, 2048*3]

    s_sb = pool_st.tile([128, KH], F32)
    e_sb = pool_st.tile([128, KH], BF16)
    gtt = pool_tiny.tile([128, 4], F32)
    # accumulators: quantity n (pairs 0-8, sumE=9), slot c -> col n*NAC+c
    aacc = pool_tiny.tile([128, 10 * NAC], F32)
    nc.gpsimd.memset(aacc[:], 0.0)

    # ---------------- DMA streams ----------------
    # SP queue: ori chunks (dense), then point chunks 2,3.
    # ACT queue: sep chunks (interleaved with exps), gtt, point chunks 0,1.
    # SP queue: ori chunks dense (chunk 0 in 2 halves), then points.
    # ACT queue: sep/exp stream (chunk 0 halved) + gtt.
    qcs = []
    half = KC * 2
    for c in range(NKC):
        qc = pool_q.tile([128, KC * 4], F32, tag=f"qc{c}")
        qcs.append(qc)
    nc.sync.dma_start(qcs[0][:, 0:half], ori_v[:, 0:half])
    nc.sync.dma_start(qcs[0][:, half:KC * 4], ori_v[:, half:KC * 4])
    for c in range(1, NKC):
        nc.sync.dma_start(qcs[c][:], ori_v[:, c * KC * 4:(c + 1) * KC * 4])
    ptcs = []
    for pc_ in range(NPC):
        ptc = pool_pt.tile([128, PC * 3], F32, tag=f"ptc{pc_}")
        ptcs.append(ptc)
    for pc_ in range(NPC):
        nc.sync.dma_start(ptcs[pc_][:], pt_v[:, pc_ * PC * 3:(pc_ + 1) * PC * 3])

    # ACT queue: dummy exp first so the exp-table load (1283ns) runs before
    # the first sep chunk lands.
    sqb = pool_tiny.tile([128, 1], F32)
    nc.gpsimd.memset(sqb[:], 0.0)
    tdum = pool_tiny.tile([128, 1], F32)
    nc.scalar.activation(tdum[:], sqb[:], ACT.Exp)
    HKC = KC // 2
    sumE0 = 9 * NAC
    for h in range(2):
        ks = slice(h * HKC, (h + 1) * HKC)
        nc.scalar.dma_start(s_sb[:, ks], sep_v[:, ks])
        nc.scalar.activation(e_sb[:, ks], s_sb[:, ks], ACT.Exp,
                             accum_out=aacc[:, sumE0 + h:sumE0 + h + 1])
    for c in range(1, NKC):
        ks = slice(c * KC, (c + 1) * KC)
        nc.scalar.dma_start(s_sb[:, ks], sep_v[:, ks])
        nc.scalar.activation(e_sb[:, ks], s_sb[:, ks], ACT.Exp,
                             accum_out=aacc[:, sumE0 + 1 + c:sumE0 + 2 + c])
        if c == 1:
            nc.scalar.dma_start(gtt[0:128:2, :], gt[:, :])
            nc.scalar.dma_start(gtt[1:128:2, :], gt[:, :])

    # ---------------- stage A: u planes + Gram pairs ----------------
    scrD = pool_pr.tile([128, KC], BF16)

    def stage_a(c, lo, w, col):
        """Emit u + 9 pair ops for chunk c covering k in [lo, lo+w)."""
        qcv = qcs[c][:, lo * 4:(lo + w) * 4].rearrange("p (k i) -> p i k", i=4)
        ec = e_sb[:, c * KC + lo:c * KC + lo + w]
        if lo == 0:
            u_t = pool_u.tile([128, 3 * KC], BF16, tag=f"u{c}")
            stage_a.u_cur = u_t
        u = stage_a.u_cur
        uv = u[:].rearrange("p (i k) -> p i k", i=3)
        for i in range(3):
            eng = nc.vector if U_ENG[i] == 'D' else nc.gpsimd
            eng.tensor_tensor(uv[:, i, lo:lo + w], ec, qcv[:, i, :], op=OP.mult)
        for n, (i, j) in enumerate(PAIRS):
            acol = aacc[:, n * NAC + col:n * NAC + col + 1]
            if c == 3:
                # last chunk: DVE-heavy with inline accums and no
                # Pool->DVE->ACT relay, so the accumulate latency chain
                # doesn't extend the stage-A drain before the eigen chain
                mode = PAIR_ENG_C3[n]
            elif PAIR_ENG[n] == 'PA' and c == 2:
                # chunk 2: shorter ACT accums (fold first)
                mode = 'PF'
            else:
                mode = PAIR_ENG[n]
            if mode == 'D':
                nc.vector.scalar_tensor_tensor(
                    scrD[:, 0:w], uv[:, i, lo:lo + w], 1.0, qcv[:, j, :],
                    op0=OP.mult, op1=OP.mult, accum_out=acol)
            else:
                scr = pool_pr.tile([128, KC], BF16, tag=f"sp{n}{col % 2}")
                nc.gpsimd.tensor_tensor(scr[:, 0:w], uv[:, i, lo:lo + w],
                                        qcv[:, j, :], op=OP.mult)
                if mode == 'PF' and w >= KC:
                    hw_ = w // 2
                    scr2 = pool_pr.tile([128, KC // 2], BF16,
                                        tag=f"sf{n}{col % 2}")
                    nc.vector.tensor_tensor(scr2[:, 0:hw_], scr[:, 0:hw_],
                                            scr[:, hw_:w], op=OP.add)
                    nc.scalar.activation(scr2[:, 0:hw_], scr2[:, 0:hw_],
                                         ACT.Identity, accum_out=acol)
                else:
                    nc.scalar.activation(scr[:, 0:w], scr[:, 0:w],
                                         ACT.Identity, accum_out=acol)

    stage_a(0, 0, HKC, 0)
    stage_a(0, HKC, HKC, 1)
    stage_a(1, 0, KC, 2)
    stage_a(2, 0, KC, 3)
    stage_a(3, 0, KC, 4)

    # ---------------- L(gt) build (tiny, off the critical path) -------------
    Lm = pool_tiny.tile([128, 12], F32)
    ngt = pool_tiny.tile([128, 4], F32)
    nc.vector.tensor_scalar(ngt[:], gtt[:], -1.0, None, op0=OP.mult)
    lsrc = [(0, 1, True), (1, 0, False), (2, 3, True), (3, 2, False),
            (4, 2, True), (5, 3, False), (6, 0, False), (7, 1, True),
            (8, 3, True), (9, 2, True), (10, 1, False), (11, 0, False)]
    for idx, (dst, src_i, neg) in enumerate(lsrc):
        src = ngt if neg else gtt
        eng = nc.gpsimd if idx % 2 == 0 else nc.vector
        eng.tensor_copy(Lm[:, dst:dst + 1], src[:, src_i:src_i + 1])
    x0t = pool_tiny.tile([128, 4], F32)
    for j, val in enumerate([1.0, 0.61, 0.37, 0.22]):
        nc.gpsimd.memset(x0t[:, j:j + 1], val)

    # ---------------- stage C prep: |p|^2 planes (overlaps stage A) ---------
    # squares: ACT (Square is in the exp table set) for c0/c1 as deint
    # planes; Pool interleaved TT squares for c2/c3. Plane adds: DVE packed
    # (c0/c1), Pool strided (c2/c3). The sqrt-set load is hoisted right
    # after the last square, off the critical tail.
    sspl = pool_tiny.tile([128, PH], BF16)
    pn = pool_tiny.tile([128, NPC], F32)
    sqts = []
    for c in range(NPC):
        sq = pool_c.tile([128, 3 * PC], BF16, tag=f"sq{c % 2}")
        if c < 2:
            pv = ptcs[c][:].rearrange("p (k i) -> p i k", i=3)
            sqv = sq[:].rearrange("p (i k) -> p i k", i=3)
            for i in range(3):
                nc.scalar.activation(sqv[:, i, :], pv[:, i, :], ACT.Square)
        else:
            nc.gpsimd.tensor_tensor(sq[:], ptcs[c][:], ptcs[c][:], op=OP.mult)
        sqts.append(sq)
    tdum2 = pool_tiny.tile([128, 1], F32)
    nc.scalar.activation(tdum2[:], sqb[:], ACT.Sqrt)
    for c in range(NPC):
        cs = slice(c * PC, (c + 1) * PC)
        s01 = pool_c.tile([128, PC], BF16, tag=f"s01{c % 2}")
        if c < 2:
            sqv = sqts[c][:].rearrange("p (i k) -> p i k", i=3)
            nc.vector.tensor_tensor(s01[:], sqv[:, 0, :], sqv[:, 1, :],
                                    op=OP.add)
            nc.vector.tensor_tensor(sspl[:, cs], s01[:], sqv[:, 2, :],
                                    op=OP.add)
        else:
            sqv = sqts[c][:].rearrange("p (k i) -> p k i", i=3)
            nc.gpsimd.tensor_tensor(s01[:], sqv[:, :, 0], sqv[:, :, 1],
                                    op=OP.add)
            nc.gpsimd.tensor_tensor(sspl[:, cs], s01[:], sqv[:, :, 2],
                                    op=OP.add)

    # ---------------- Gram combine: chunks then even/odd halves -------------
    # pair-sum over (2s, 2s+1) partitions without DMA latency: 32x32 block
    # stream-transpose -> free-axis pair add -> transpose back.
    r32 = pool_tiny.tile([128, 32], F32)
    nc.gpsimd.memset(r32[:], 0.0)
    nc.vector.tensor_reduce(r32[:, 0:10],
                            aacc[:].rearrange("p (n c) -> p n c", c=NAC),
                            axis=AX.X, op=OP.add)
    tT = pool_tiny.tile([128, 32], F32)
    nc.vector.transpose(tT[:], r32[:])
    d32 = pool_tiny.tile([128, 32], F32)
    tv = tT[:].rearrange("p (m t) -> p m t", t=2)
    nc.vector.tensor_tensor(
        d32[:].rearrange("p (m t) -> p m t", t=2),
        tv[:, :, 0:1].broadcast_to([128, 16, 2]),
        tv[:, :, 1:2].broadcast_to([128, 16, 2]), op=OP.add)
    a32 = pool_tiny.tile([128, 32], F32)
    nc.vector.transpose(a32[:], d32[:])
    a10 = a32
    # A33 = sumE - A00 - A11 - A22   (a10 col 9 holds sumE)
    for d in ((0, 0), (1, 1), (2, 2)):
        nc.vector.tensor_tensor(a10[:, 9:10], a10[:, 9:10],
                                a10[:, UIDX[d]:UIDX[d] + 1], op=OP.subtract)

    # full 4x4 matrix [128, 16] row-major; slab copies split DVE/gpsimd
    amat = pool_tiny.tile([128, 16], F32)
    copies = [  # (dst_col, src_col, width)
        (0, 0, 4), (4, 1, 1), (5, 4, 3), (8, 2, 1), (9, 5, 1), (10, 7, 2),
        (12, 3, 1), (13, 6, 1), (14, 8, 2),
    ]
    for idx, (dc, sc_, w) in enumerate(copies):
        eng = nc.vector if idx % 2 == 0 else nc.gpsimd
        eng.tensor_copy(amat[:, dc:dc + w], a10[:, sc_:sc_ + w])

    # ---------------- stage B: matrix squaring (A -> A^(2^NSQ)) -------------
    a_cur = amat
    trv = pool_tiny.tile([128, 1], F32)
    tri = pool_tiny.tile([128, 1], F32)
    t64 = pool_tiny.tile([128, 64], F32)
    for m in range(NSQ):
        a_new = pool_tiny.tile([128, 16], F32, tag=f"asq{m % 2}")
        in0 = a_cur[:].rearrange("p (i j) -> p i j", j=4).unsqueeze(2) \
                      .broadcast_to([128, 4, 4, 4])
        in1 = a_cur[:].rearrange("p (j k) -> p k j", k=4).unsqueeze(1) \
                      .broadcast_to([128, 4, 4, 4])
        nc.vector.tensor_tensor(
            t64[:].rearrange("p (i k j) -> p i k j", k=4, j=4), in0, in1,
            op=OP.mult)
        nc.vector.tensor_reduce(
            a_new[:].rearrange("p (i k) -> p i k", k=4),
            t64[:].rearrange("p (ik j) -> p ik j", j=4), axis=AX.X, op=OP.add)
        if m in NORM_AT:
            nc.vector.tensor_reduce(trv[:], a_new[:, 0::5], axis=AX.X, op=OP.add)
            nc.vector.reciprocal(tri[:], trv[:])
            nc.vector.tensor_scalar(a_new[:], a_new[:], tri[:], None, op0=OP.mult)
        a_cur = a_new

    # h = A^N @ x0 ; vp = L @ h ; per-sample scalars
    t16 = pool_tiny.tile([128, 16], F32)
    h4 = pool_tiny.tile([128, 4], F32)
    nc.vector.tensor_tensor(t16[:].rearrange("p (i j) -> p i j", j=4),
                            a_cur[:].rearrange("p (i j) -> p i j", j=4),
                            x0t[:].unsqueeze(1).broadcast_to([128, 4, 4]),
                            op=OP.mult)
    nc.vector.tensor_reduce(h4[:].unsqueeze(2),
                            t16[:].rearrange("p (i j) -> p i j", j=4),
                            axis=AX.X, op=OP.add)
    scB = pool_tiny.tile([128, 5], F32)   # vpx,vpy,vpz,negr,c1
    t12 = pool_tiny.tile([128, 12], F32)
    nc.vector.tensor_tensor(t12[:].rearrange("p (r j) -> p r j", j=4),
                            Lm[:].rearrange("p (r j) -> p r j", j=4),
                            h4[:].unsqueeze(1).broadcast_to([128, 3, 4]),
                            op=OP.mult)
    nc.vector.tensor_reduce(scB[:, 0:3].unsqueeze(2),
                            t12[:].rearrange("p (r j) -> p r j", j=4),
                            axis=AX.X, op=OP.add)
    hv2 = pool_tiny.tile([128, 4], F32)
    vv2 = pool_tiny.tile([128, 1], F32)
    nc.vector.scalar_tensor_tensor(hv2[:], h4[:], 1.0, h4[:],
                                   op0=OP.mult, op1=OP.mult, accum_out=vv2[:])
    vp2 = pool_tiny.tile([128, 1], F32)
    sq3 = pool_tiny.tile([128, 3], F32)
    nc.vector.tensor_tensor(sq3[:], scB[:, 0:3], scB[:, 0:3], op=OP.mult)
    nc.vector.tensor_reduce(vp2[:], sq3[:], axis=AX.X, op=OP.add)
    nvp2 = pool_tiny.tile([128, 1], F32)
    nc.vector.tensor_scalar(nvp2[:], vp2[:], -1.0, None, op0=OP.mult)
    nc.vector.reciprocal(scB[:, 3:4], nvp2[:])          # negr = -1/|vp|^2
    c1t = pool_tiny.tile([128, 1], F32)
    nc.vector.reciprocal(c1t[:], vv2[:])
    nc.vector.tensor_tensor(c1t[:], c1t[:], vp2[:], op=OP.mult)
    nc.vector.tensor_scalar(scB[:, 4:5], c1t[:], 4.0, None, op0=OP.mult)

    # ---------------- stage C finish: distances -----------------------------
    # d = vp . p per point; chunk chains on DVE (c0,c1) and Pool (c2,c3),
    # two chunks interleaved per engine so the serial per-chunk deps hide.
    # chains: DVE STT for c0/c1; Pool TT with broadcast per-sample scalars
    # for c2/c3 (Pool has no TensorScalarPtr); relu on DVE (4x mode).
    pvs = [ptcs[c][:].rearrange("p (k i) -> p i k", i=3) for c in range(NPC)]
    s2s = {}
    for c in (0, 1):
        dt = pool_c.tile([128, PC], BF16, tag=f"dt{c}")
        nc.vector.tensor_scalar(dt[:], pvs[c][:, 0, :], scB[:, 0:1], None,
                                op0=OP.mult)
        d = pool_c.tile([128, PC], BF16, tag=f"d{c}")
        nc.vector.scalar_tensor_tensor(d[:], pvs[c][:, 1, :], scB[:, 1:2],
                                       dt[:], op0=OP.mult, op1=OP.add)
        nc.vector.scalar_tensor_tensor(d[:], pvs[c][:, 2, :], scB[:, 2:3],
                                       d[:], op0=OP.mult, op1=OP.add)
        dd = pool_c.tile([128, PC], BF16, tag=f"dd{c}")
        nc.vector.tensor_tensor(dd[:], d[:], d[:], op=OP.mult)
        s2 = pool_c.tile([128, PC], BF16, tag=f"s2{c}")
        nc.vector.scalar_tensor_tensor(s2[:], dd[:], scB[:, 3:4],
                                       sspl[:, c * PC:(c + 1) * PC],
                                       op0=OP.mult, op1=OP.add)
        s2s[c] = s2
    for c in (2, 3):
        bvp = [scB[:, i:i + 1].broadcast_to([128, PC]) for i in range(4)]
        dt = pool_c.tile([128, PC], BF16, tag=f"dt{c}")
        nc.gpsimd.tensor_tensor(dt[:], pvs[c][:, 0, :], bvp[0], op=OP.mult)
        d1 = pool_c.tile([128, PC], BF16, tag=f"d1{c}")
        nc.gpsimd.tensor_tensor(d1[:], pvs[c][:, 1, :], bvp[1], op=OP.mult)
        nc.gpsimd.tensor_tensor(d1[:], d1[:], dt[:], op=OP.add)
        d2 = pool_c.tile([128, PC], BF16, tag=f"d2{c}")
        nc.gpsimd.tensor_tensor(d2[:], pvs[c][:, 2, :], bvp[2], op=OP.mult)
        nc.gpsimd.tensor_tensor(d1[:], d1[:], d2[:], op=OP.add)
        dd = pool_c.tile([128, PC], BF16, tag=f"dd{c}")
        nc.gpsimd.tensor_tensor(dd[:], d1[:], d1[:], op=OP.mult)
        s2 = pool_c.tile([128, PC], BF16, tag=f"s2{c}")
        if c == 3:
            # last chunk: finish on DVE (idle by then) to shorten the Pool
            # chain stream that gates the final sqrt
            nc.vector.scalar_tensor_tensor(s2[:], dd[:], scB[:, 3:4],
                                           sspl[:, c * PC:(c + 1) * PC],
                                           op0=OP.mult, op1=OP.add)
        else:
            nc.gpsimd.tensor_tensor(dd[:], dd[:], bvp[3], op=OP.mult)
            nc.gpsimd.tensor_tensor(s2[:], dd[:],
                                    sspl[:, c * PC:(c + 1) * PC], op=OP.add)
        s2s[c] = s2
    for c in (0, 1, 2, 3):
        nc.vector.tensor_scalar(s2s[c][:], s2s[c][:], 0.0, None, op0=OP.max)
    for pos, c in enumerate((0, 2, 1, 3)):
        sqo = pool_c.tile([128, PC], BF16, tag=f"sqo{c}")
        nc.scalar.activation(sqo[:], s2s[c][:], ACT.Sqrt, bias=sqb[:],
                             scale=scB[:, 4:5], accum_out=pn[:, pos:pos + 1])

    # pn [128, NPC] goes straight to DRAM in two halves so the first DMA's
    # latency overlaps the second side's sqrts; the host sums the 4 columns
    nc.sync.dma_start(out[:, 0:2], pn[:, 0:2])
    nc.sync.dma_start(out[:, 2:4], pn[:, 2:4])


_NC_CACHE = {}


def _build():
    if "nc" in _NC_CACHE:
        return _NC_CACHE["nc"]
    nc = bacc.Bacc("TRN2", target_bir_lowering=False, debug=False,
                   enable_asserts=True, num_devices=NCORES)
    sep = nc.declare_dram_parameter("softEncodePred", [S, K], F32, isOutput=False)
    ori = nc.declare_dram_parameter("oriHistogramMap", [S, K, 4], F32, isOutput=False)
    gt = nc.declare_dram_parameter("gt", [S, 4], F32, isOutput=False)
    pt = nc.declare_dram_parameter("point", [S, P, 3], F32, isOutput=False)
    out = nc.declare_dram_parameter("out", [128, NPC], F32, isOutput=True)
    with tile.TileContext(nc) as tc:
        with ExitStack() as ctx:
            _emit(ctx, tc, sep.ap(), ori.ap(), gt.ap(), pt.ap(), out.ap())
    nc.finalize()
    _NC_CACHE["nc"] = nc
    return nc


def kernel(softEncodePred, oriHistogramMap, gt, point):
    nc = _build()
    in_maps = []
    for c in range(NCORES):
        sl = slice(c * S, (c + 1) * S)
        in_maps.append({
            "softEncodePred": np.ascontiguousarray(softEncodePred[sl], np.float32),
            "oriHistogramMap": np.ascontiguousarray(oriHistogramMap[sl], np.float32),
            "gt": np.ascontiguousarray(gt[sl], np.float32),
            "point": np.ascontiguousarray(point[sl], np.float32),
        })
    from concourse.bass_utils import run_bass_kernel_spmd
    res = run_bass_kernel_spmd(nc, in_maps, core_ids=list(range(NCORES)))
    total = np.float64(0.0)
    for r in res.results:
        total += np.asarray(r["out"], np.float64).sum()
    return np.float32(total / (B * P))
